# revision 2
# baseline (speedup 1.0000x reference)
"""Trainium2 Bass kernel for nn_Downsample (depthwise 4x4 FIR, stride 2).

Strategy: data-parallel over batch (8 cores, one batch element each).
Separable FIR downsample split across engines to hit the DMA roofline:

  Stage 1 (H-downsample) on the tensor engine: the tiny polyphase band
  matrices A_e/A_o [128,128] are the STATIONARY operand (loaded rarely);
  x streams through as the moving operand with N=512 columns per matmul.
  Host pre-transposes x to [phase, h/2, c, w] so every DMA descriptor is
  fully contiguous per partition (~32 KB runs).

  Stage 2 (W-downsample) as a strided 4-tap FIR straight out of PSUM,
  split between the scalar engine (tap 1 via activation-copy-with-scale)
  and the vector engine (taps 2, 0, 3 via scalar_tensor_tensor MACs).
  The pair-split PSUM view [p, c, w', 2] gives stride-2 taps with plain
  slicing; edge columns are handled by sub-range ops plus one batched
  tail copy per channel block.

fp16 input/output halves DMA bytes; accumulation stays fp32 in PSUM.
"""

import numpy as np

B, C, H, W = 8, 256, 256, 256
HO, WO = H // 2, W // 2
N_CORES = 8
TAPS = 4
PAD0 = 1          # (kh - factor + 1) // 2 for kh=4, factor=2
CB = 32           # channels per DMA block
PT = 8            # channels per PSUM tile (4 banks, 2 ch/bank)

_CACHE = {}


def _band_matrix(g, n_in, n_out):
    """A[h, h'] = g[i] at h = 2*h' - PAD0 + i, zero-padded at the edges."""
    a = np.zeros((n_in, n_out), dtype=np.float32)
    for hp in range(n_out):
        for i in range(TAPS):
            h = 2 * hp - PAD0 + i
            if 0 <= h < n_in:
                a[h, hp] = g[i]
    return a


def _build_program():
    from concourse import bacc, tile
    import concourse.mybir as mybir

    R = mybir.dt.float16
    F32 = mybir.dt.float32
    MULT = mybir.AluOpType.mult
    ADD = mybir.AluOpType.add
    COPY = mybir.ActivationFunctionType.Copy

    nc = bacc.Bacc("TRN2", target_bir_lowering=False, debug=False,
                   num_devices=N_CORES)
    # host-prepped layouts: x (ph, h', c, w); amat (ph, h_half, h'); gw bcast
    x_d = nc.dram_tensor("x", [2, HO, C, W], R, kind="ExternalInput").ap()
    am_d = nc.dram_tensor("amat", [2, HO, HO], R, kind="ExternalInput").ap()
    gw_d = nc.dram_tensor("gw", [128, TAPS], F32, kind="ExternalInput").ap()
    y_d = nc.dram_tensor("y", [HO, C, WO], R, kind="ExternalOutput").ap()

    n_cblk = C // CB
    n_pt = CB // PT

    with tile.TileContext(nc) as tc:
        with tc.tile_pool(name="const", bufs=1) as constp, \
             tc.tile_pool(name="xin", bufs=3) as xinp, \
             tc.tile_pool(name="accp", bufs=2) as accp, \
             tc.tile_pool(name="outp", bufs=2) as outp, \
             tc.tile_pool(name="ps", bufs=2, space="PSUM") as psp:

            am_t = constp.tile([128, 2, HO], R)
            nc.sync.dma_start(out=am_t[:], in_=am_d.rearrange("k p m -> p k m"))
            gw_t = constp.tile([128, TAPS], F32)
            nc.sync.dma_start(out=gw_t[:], in_=gw_d)

            for cb in range(n_cblk):
                c0 = cb * CB
                xt = xinp.tile([128, 2, CB, W], R, tag="x")
                nc.sync.dma_start(
                    out=xt[:],
                    in_=x_d[:, :, c0:c0 + CB, :].rearrange("k p c w -> p k c w"))
                acc = accp.tile([128, CB, WO], F32, tag="acc")
                ot = outp.tile([128, CB, WO], R, tag="out")

                for pt in range(n_pt):
                    s0 = pt * PT
                    ps = psp.tile([128, PT, W], F32)
                    for bk in range(PT // 2):
                        cc = s0 + 2 * bk
                        dst = ps[:, 2 * bk:2 * bk + 2, :]
                        for ph in range(2):
                            nc.tensor.matmul(
                                dst, am_t[:, ph, :], xt[:, ph, cc:cc + 2, :],
                                start=(ph == 0), stop=(ph == 1))

                    # pair-split view: v[p, c, j, par] = psum col (2j+par)
                    v = ps[:].rearrange("p c (j two) -> p c j two", two=2)
                    a = acc[:, s0:s0 + PT, :]
                    o = ot[:, s0:s0 + PT, :]
                    # tap1 (s=2w'):      a  = g1 * v[...,0]           (ACT)
                    nc.scalar.activation(a, v[:, :, :, 0], COPY,
                                         scale=gw_t[:, 1:2])
                    # tap2 (s=2w'+1):    a += g2 * v[...,1]           (DVE)
                    nc.vector.scalar_tensor_tensor(
                        a, v[:, :, :, 1], gw_t[:, 2:3], a, MULT, ADD)
                    # tap0 (s=2w'-1):    a[1:] += g0 * v[...,:-1,1]   (DVE)
                    nc.vector.scalar_tensor_tensor(
                        a[:, :, 1:WO], v[:, :, 0:WO - 1, 1], gw_t[:, 0:1],
                        a[:, :, 1:WO], MULT, ADD)
                    # tap3 (s=2w'+2):    o[:-1] = g3 * v[...,1:,0] + a (DVE)
                    nc.vector.scalar_tensor_tensor(
                        o[:, :, 0:WO - 1], v[:, :, 1:WO, 0], gw_t[:, 3:4],
                        a[:, :, 0:WO - 1], MULT, ADD)

                # batched tail: w'=127 has no tap3 contribution
                nc.scalar.copy(ot[:, :, WO - 1:WO], acc[:, :, WO - 1:WO])
                nc.sync.dma_start(out=y_d[:, c0:c0 + CB, :], in_=ot[:])

    nc.compile()
    return nc


def _get_program():
    if "nc" not in _CACHE:
        _CACHE["nc"] = _build_program()
    return _CACHE["nc"]


def kernel(x, kernel):
    from concourse.bass_utils import run_bass_kernel_spmd

    x = np.asarray(x, dtype=np.float32)
    k = np.asarray(kernel, dtype=np.float32)

    # reference correlates with the flipped kernel; separable factors from
    # row/col sums (exact for normalized separable kernels)
    w = k[::-1, ::-1].astype(np.float64)
    g_h = w.sum(axis=1)
    g_w = w.sum(axis=0)
    s = w.sum()
    if not np.isclose(s, 1.0):
        g_h = g_h / np.sqrt(s)
        g_w = g_w / np.sqrt(s)
    g_h = g_h.astype(np.float32)
    g_w = g_w.astype(np.float32)

    a_h = _band_matrix(g_h, H, HO)                       # [H, HO]
    amat = np.ascontiguousarray(
        a_h.reshape(HO, 2, HO).transpose(1, 0, 2)).astype(np.float16)
    gw_host = np.ascontiguousarray(
        np.broadcast_to(g_w[None, :], (128, TAPS))).astype(np.float32)

    nc = _get_program()
    in_maps = []
    for b in range(B):
        xb = x[b].astype(np.float16)                     # [C, H, W]
        xp = np.ascontiguousarray(
            xb.transpose(1, 0, 2).reshape(HO, 2, C, W).transpose(1, 0, 2, 3))
        in_maps.append({"x": xp, "amat": amat, "gw": gw_host})

    res = run_bass_kernel_spmd(nc, in_maps, core_ids=list(range(N_CORES)))
    _CACHE["last_result"] = res
    out = np.stack(
        [res.results[b]["y"].transpose(1, 0, 2) for b in range(B)], axis=0)
    return out.astype(np.float32)


# revision 4
# speedup vs baseline: 1.5373x; 1.5373x over previous
"""Trainium2 Bass kernel for nn_Downsample (depthwise 4x4 FIR, stride 2).

Strategy: data-parallel over batch (8 cores, one batch element each).
Separable FIR downsample split across engines to hit the DMA roofline:

  Stage 1 (H-downsample) on the tensor engine: the tiny polyphase band
  matrices A_e/A_o [128,128] are the STATIONARY operand; x streams
  through as the moving operand with N=512 columns per matmul, grouped
  4 matmuls per weight load.  The host pre-transposes x to
  [h-phase, h/2, c, w-parity, w/2] so (a) every DMA descriptor is fully
  contiguous per partition (~32 KB runs) and (b) the stage-1 PSUM output
  lands with even/odd W columns in separate contiguous blocks.

  Stage 2 (W-downsample): ACT copies the two parity blocks PSUM->SBUF
  fp16 with the inner tap weights (g1, g2) folded into the copy scale;
  DVE then needs only 3 step-1 fp16 ops per tile: add, plus two
  scalar_tensor_tensor MACs with ratio scalars g0/g2 and g3/g1.  Edge
  columns are sub-range ops plus one batched tail copy per block.

fp16 input/output halves DMA bytes; stage-1 accumulation is fp32 PSUM.
"""

import numpy as np

B, C, H, W = 8, 256, 256, 256
HO, WO = H // 2, W // 2
N_CORES = 8
TAPS = 4
PAD0 = 1          # (kh - factor + 1) // 2 for kh=4, factor=2
CB = 32           # channels per DMA block
PT = 8            # channels per PSUM tile (4 banks, 2 ch/bank)

_CACHE = {}


def _band_matrix(g, n_in, n_out):
    """A[h, h'] = g[i] at h = 2*h' - PAD0 + i, zero-padded at the edges."""
    a = np.zeros((n_in, n_out), dtype=np.float32)
    for hp in range(n_out):
        for i in range(TAPS):
            h = 2 * hp - PAD0 + i
            if 0 <= h < n_in:
                a[h, hp] = g[i]
    return a


def _build_program(fast):
    from concourse import bacc, tile
    import concourse.mybir as mybir

    R = mybir.dt.float16
    F32 = mybir.dt.float32
    MULT = mybir.AluOpType.mult
    ADD = mybir.AluOpType.add
    COPY = mybir.ActivationFunctionType.Copy

    nc = bacc.Bacc("TRN2", target_bir_lowering=False, debug=False,
                   num_devices=N_CORES)
    # host-prepped layouts; the W axis of x is (w-parity, w/2)
    x_d = nc.dram_tensor("x", [2, HO, C, W], R, kind="ExternalInput").ap()
    am_d = nc.dram_tensor("amat", [2, HO, HO], R, kind="ExternalInput").ap()
    gw_d = nc.dram_tensor("gw", [128, TAPS], F32, kind="ExternalInput").ap()
    # sc: per-partition copy scales [s_e, s_o]
    sc_d = nc.dram_tensor("sc", [128, 2], F32, kind="ExternalInput").ap()
    y_d = nc.dram_tensor("y", [HO, C, WO], R, kind="ExternalOutput").ap()

    n_cblk = C // CB
    n_pt = CB // PT

    with tile.TileContext(nc) as tc:
        with tc.tile_pool(name="const", bufs=1) as constp, \
             tc.tile_pool(name="xin", bufs=3) as xinp, \
             tc.tile_pool(name="sep", bufs=4) as sepp, \
             tc.tile_pool(name="accp", bufs=2) as accp, \
             tc.tile_pool(name="outp", bufs=2) as outp, \
             tc.tile_pool(name="ps", bufs=2, space="PSUM") as psp:

            am_t = constp.tile([128, 2, HO], R)
            nc.sync.dma_start(out=am_t[:], in_=am_d.rearrange("k p m -> p k m"))
            gw_t = constp.tile([128, TAPS], F32)
            nc.sync.dma_start(out=gw_t[:], in_=gw_d)
            sc_t = constp.tile([128, 2], F32)
            nc.sync.dma_start(out=sc_t[:], in_=sc_d)

            for cb in range(n_cblk):
                c0 = cb * CB
                xt = xinp.tile([128, 2, CB, W], R, tag="x")
                nc.sync.dma_start(
                    out=xt[:],
                    in_=x_d[:, :, c0:c0 + CB, :].rearrange("k p c w -> p k c w"))
                acc = accp.tile([128, CB, WO], R, tag="acc")
                ot = outp.tile([128, CB, WO], R, tag="out")

                for pt in range(n_pt):
                    s0 = pt * PT
                    # psum cols per channel: (w-parity, w')
                    ps = psp.tile([128, PT, 2, WO], F32)
                    for ph in range(2):
                        for bk in range(PT // 2):
                            cc = s0 + 2 * bk
                            nc.tensor.matmul(
                                ps[:, 2 * bk:2 * bk + 2, :, :],
                                am_t[:, ph, :], xt[:, ph, cc:cc + 2, :],
                                start=(ph == 0), stop=(ph == 1),
                                skip_group_check=True)

                    ev = ps[:, :, 0, :]           # T[2w']  (taps 1, 3)
                    od = ps[:, :, 1, :]           # T[2w'+1] (taps 2, 0)
                    se = sepp.tile([128, PT, WO], R, tag="se")
                    so = sepp.tile([128, PT, WO], R, tag="so")
                    a = acc[:, s0:s0 + PT, :]
                    o = ot[:, s0:s0 + PT, :]
                    # ACT: parity copies with inner tap weights folded in
                    nc.scalar.activation(se[:], ev, COPY, scale=sc_t[:, 0:1])
                    nc.scalar.activation(so[:], od, COPY, scale=sc_t[:, 1:2])
                    if fast:
                        # a = g1*T1 + g2*T2 ; se/so arrive pre-scaled
                        nc.vector.tensor_add(a, se[:], so[:])
                    else:
                        # se/so unscaled: a = (se*g1) + (so*g2)
                        nc.vector.tensor_scalar_mul(a, so[:], gw_t[:, 2:3])
                        nc.vector.scalar_tensor_tensor(
                            a, se[:], gw_t[:, 1:2], a, MULT, ADD)
                    # tap0: a[1:] += r0 * so[:-1]
                    nc.vector.scalar_tensor_tensor(
                        a[:, :, 1:WO], so[:, :, 0:WO - 1], gw_t[:, 0:1],
                        a[:, :, 1:WO], MULT, ADD)
                    # tap3: o[:-1] = r3 * se[1:] + a[:-1]
                    nc.vector.scalar_tensor_tensor(
                        o[:, :, 0:WO - 1], se[:, :, 1:WO], gw_t[:, 3:4],
                        a[:, :, 0:WO - 1], MULT, ADD)

                # batched tail: w'=127 has no tap3 contribution
                nc.scalar.copy(ot[:, :, WO - 1:WO], acc[:, :, WO - 1:WO])
                nc.sync.dma_start(out=y_d[:, c0:c0 + CB, :], in_=ot[:])

    nc.compile()
    return nc


def _get_program(fast=True):
    key = ("nc", fast)
    if key not in _CACHE:
        _CACHE[key] = _build_program(fast)
    return _CACHE[key]


def kernel(x, kernel):
    from concourse.bass_utils import run_bass_kernel_spmd

    x = np.asarray(x, dtype=np.float32)
    k = np.asarray(kernel, dtype=np.float32)

    # reference correlates with the flipped kernel; separable factors from
    # row/col sums (exact for normalized separable kernels)
    w = k[::-1, ::-1].astype(np.float64)
    g_h = w.sum(axis=1)
    g_w = w.sum(axis=0)
    s = w.sum()
    if not np.isclose(s, 1.0):
        g_h = g_h / np.sqrt(s)
        g_w = g_w / np.sqrt(s)
    g_h = g_h.astype(np.float32)
    g_w = g_w.astype(np.float32)

    tol = 1e-6 * max(1.0, float(np.abs(g_w).max()))
    fast = abs(float(g_w[1])) > tol and abs(float(g_w[2])) > tol
    if fast:
        scales = np.array([g_w[1], g_w[2]], dtype=np.float32)
        gvals = np.array([g_w[0] / g_w[2], g_w[1], g_w[2], g_w[3] / g_w[1]],
                         dtype=np.float32)
    else:
        scales = np.array([1.0, 1.0], dtype=np.float32)
        gvals = g_w

    a_h = _band_matrix(g_h, H, HO)                       # [H, HO]
    amat = np.ascontiguousarray(
        a_h.reshape(HO, 2, HO).transpose(1, 0, 2)).astype(np.float16)
    gw_host = np.ascontiguousarray(
        np.broadcast_to(gvals[None, :], (128, TAPS))).astype(np.float32)
    sc_host = np.ascontiguousarray(
        np.broadcast_to(scales[None, :], (128, 2))).astype(np.float32)

    nc = _get_program(fast)
    in_maps = []
    for b in range(B):
        xb = x[b].astype(np.float16)                     # [C, H, W]
        # -> [h-phase, h/2, c, w-parity, w/2], flattened W axis
        xp = np.ascontiguousarray(
            xb.reshape(C, HO, 2, WO, 2).transpose(2, 1, 0, 4, 3)
        ).reshape(2, HO, C, W)
        in_maps.append({"x": xp, "amat": amat, "gw": gw_host, "sc": sc_host})

    res = run_bass_kernel_spmd(nc, in_maps, core_ids=list(range(N_CORES)))
    _CACHE["last_result"] = res
    out = np.stack(
        [res.results[b]["y"].transpose(1, 0, 2) for b in range(B)], axis=0)
    return out.astype(np.float32)


# revision 16
# speedup vs baseline: 1.5441x; 1.0044x over previous
"""Trainium2 Bass kernel for nn_Downsample (depthwise 4x4 FIR, stride 2).

Strategy: data-parallel over batch (8 cores, one batch element each).
The whole separable FIR runs on the tensor engine as a sum of 8 band-matrix
matmuls per PSUM region:

  out[h',c,w'] = sum_{i,ph} (g_w[i] * A_ph)^T  X[ph, :, c, par(i), w'+off(i)]

The host pre-transposes x to [h-phase, h/2, c, w-parity, w/2] so that
every DMA descriptor is fully contiguous per partition (~32 KB runs) and
each W-tap of the output is an aligned (or 1-shifted) slice of the moving
operand.  The H-FIR lives in the polyphase band matrices A_e/A_o
[128x128]; the W-FIR taps become 8 pre-scaled stationary matrices
(g_w[i] * A_ph) accumulated into the same PSUM region, with the two
shifted taps writing partial column ranges (which also handles the W
edges exactly).  Stage 2 is then just a PSUM -> SBUF fp16 copy,
alternated between the scalar and vector engines.

fp16 input/output halves DMA bytes; accumulation is fp32 in PSUM.
"""

import numpy as np

B, C, H, W = 8, 256, 256, 256
HO, WO = H // 2, W // 2
N_CORES = 8
TAPS = 4
PAD0 = 1          # (kh - factor + 1) // 2 for kh=4, factor=2
CB = 32           # channels per DMA block
PT = 8            # channels per PSUM tile (2 banks, 4 ch/bank)

_CACHE = {}


def _band_matrix(g, n_in, n_out):
    """A[h, h'] = g[i] at h = 2*h' - PAD0 + i, zero-padded at the edges."""
    a = np.zeros((n_in, n_out), dtype=np.float32)
    for hp in range(n_out):
        for i in range(TAPS):
            h = 2 * hp - PAD0 + i
            if 0 <= h < n_in:
                a[h, hp] = g[i]
    return a


def _build_program():
    from concourse import bacc, tile
    import concourse.mybir as mybir

    R = mybir.dt.float16
    F32 = mybir.dt.float32

    nc = bacc.Bacc("TRN2", target_bir_lowering=False, debug=False,
                   num_devices=N_CORES)
    # host-prepped layouts: x (h-phase, h/2, c, w-parity, w/2)
    x_d = nc.dram_tensor("x", [2, HO, C, 2, WO], R, kind="ExternalInput").ap()
    am_d = nc.dram_tensor("amat", [8, HO, HO], R, kind="ExternalInput").ap()
    y_d = nc.dram_tensor("y", [HO, C, WO], R, kind="ExternalOutput").ap()

    n_cblk = C // CB
    n_pt = CB // PT

    # W-tap schedule: (stationary idx s = 2*i + ph, h-phase, w-parity,
    #                  out w' range, in w' range)
    # tap i=1 first (start=True, full range), tap i=2 last (stop=True, full).
    FULL = (0, WO)
    MM_PLAN = [
        (2, 0, 0, FULL, FULL),             # i=1: T_even aligned
        (3, 1, 0, FULL, FULL),
        (0, 0, 1, (1, WO), (0, WO - 1)),   # i=0: T_odd shifted left
        (1, 1, 1, (1, WO), (0, WO - 1)),
        (6, 0, 0, (0, WO - 1), (1, WO)),   # i=3: T_even shifted right
        (7, 1, 0, (0, WO - 1), (1, WO)),
        (4, 0, 1, FULL, FULL),             # i=2: T_odd aligned
        (5, 1, 1, FULL, FULL),
    ]

    with tile.TileContext(nc) as tc:
        with tc.tile_pool(name="const", bufs=1) as constp, \
             tc.tile_pool(name="xin", bufs=3) as xinp, \
             tc.tile_pool(name="outp", bufs=3) as outp, \
             tc.tile_pool(name="ps", bufs=4, space="PSUM") as psp:

            am_t = constp.tile([128, 8, HO], R)
            nc.sync.dma_start(out=am_t[:], in_=am_d.rearrange("s p m -> p s m"))

            tidx = 0
            for cb in range(n_cblk):
                c0 = cb * CB
                xt = xinp.tile([128, 2, CB, 2, WO], R, tag="x")
                nc.sync.dma_start(
                    out=xt[:],
                    in_=x_d[:, :, c0:c0 + CB, :, :].rearrange(
                        "k p c v w -> p k c v w"))
                ot = outp.tile([128, CB, WO], R, tag="out")

                for pt in range(n_pt):
                    s0 = pt * PT
                    ps = psp.tile([128, PT, WO], F32)
                    for mi, (s, ph, vv, (o0, o1), (i0, i1)) in enumerate(MM_PLAN):
                        for bk in range(PT // 4):
                            cc = s0 + 4 * bk
                            nc.tensor.matmul(
                                ps[:, 4 * bk:4 * bk + 4, o0:o1],
                                am_t[:, s, :],
                                xt[:, ph, cc:cc + 4, vv, i0:i1],
                                start=(mi == 0), stop=(mi == len(MM_PLAN) - 1),
                                skip_group_check=True)

                    # stage 2 collapsed: plain PSUM -> SBUF fp16 copy
                    if tidx % 2 == 0:
                        nc.scalar.copy(ot[:, s0:s0 + PT, :], ps[:])
                    else:
                        nc.vector.tensor_copy(ot[:, s0:s0 + PT, :], ps[:])
                    tidx += 1

                nc.sync.dma_start(out=y_d[:, c0:c0 + CB, :], in_=ot[:])

    nc.compile()
    return nc


def _get_program():
    if "nc" not in _CACHE:
        _CACHE["nc"] = _build_program()
    return _CACHE["nc"]


def kernel(x, kernel):
    from concourse.bass_utils import run_bass_kernel_spmd

    x = np.asarray(x, dtype=np.float32)
    k = np.asarray(kernel, dtype=np.float32)

    # reference correlates with the flipped kernel; separable factors from
    # row/col sums (exact for normalized separable kernels)
    w = k[::-1, ::-1].astype(np.float64)
    g_h = w.sum(axis=1)
    g_w = w.sum(axis=0)
    s = w.sum()
    if not np.isclose(s, 1.0):
        g_h = g_h / np.sqrt(s)
        g_w = g_w / np.sqrt(s)
    g_h = g_h.astype(np.float32)
    g_w = g_w.astype(np.float32)

    a_h = _band_matrix(g_h, H, HO)                       # [H, HO]
    # polyphase split: A_ph[p, m] = a_h[2p + ph, m]
    a_ph = a_h.reshape(HO, 2, HO)                        # [p, ph, m]
    amat = np.empty((8, HO, HO), dtype=np.float16)
    for i in range(TAPS):
        for ph in range(2):
            amat[2 * i + ph] = (g_w[i] * a_ph[:, ph, :]).astype(np.float16)

    nc = _get_program()
    in_maps = []
    for b in range(B):
        xb = x[b].astype(np.float16)                     # [C, H, W]
        # -> [h-phase, h/2, c, w-parity, w/2]
        xp = np.ascontiguousarray(
            xb.reshape(C, HO, 2, WO, 2).transpose(2, 1, 0, 4, 3))
        in_maps.append({"x": xp, "amat": amat})

    res = run_bass_kernel_spmd(nc, in_maps, core_ids=list(range(N_CORES)))
    _CACHE["last_result"] = res
    out = np.stack(
        [res.results[b]["y"].transpose(1, 0, 2) for b in range(B)], axis=0)
    return out.astype(np.float32)
